# revision 43
# baseline (speedup 1.0000x reference)
"""Trainium2 Bass kernel for nn_Arm_82119774699744 (dense_cnn).

Reference: 501 overlapping width-500 crops of a [B=8, 36, 1001] signal, each
through 3x (conv15-valid -> BN -> ELU -> avgpool3) -> FC(4), accumulated over
crops, /501.

Algorithm (mathematically exact, validated vs reference in fp64):
  Convs are translation-equivariant, so every crop's conv output is a slice of
  one full-width conv. Only the avgpool phase (offset mod 3) differs, giving
  3 -> 9 -> 27 phase variants of the pooled streams. Crop s maps to phase
  m = s mod 27 and offset q = s // 27. This turns ~511 GFLOP into ~11 GFLOP.

  Host-side folds: BN into conv weights/bias; avgpool mean -> sum-of-3 with
  1/3 folded into next weights; ELU stored as elu+1 = relu(z) + min(exp(z), 1)
  with the -1 folded into the next layer's bias (rowsum of its weights).

  Final FC + crop-sum + /501 folded into per-u-offset weights G applied on the
  PE: out[o] = sum_{i3,u} G_cls(i3)[o,c,u] * p3[c, 30*i3+u] where
  G_A[o,c,u] = sum_j wfc3[o,c,j] over the valid q-window (class A: q<=18,
  class B: q<=17, phase m=1 excludes q=0). Main matmuls apply G_A to all 27
  phases at once (rhs stride-30 view, N=27); class-B and m=1 corrections
  subtract the difference terms. Garbage p3 columns (u beyond a phase's valid
  range) receive exactly cancelling +/- weights, so they contribute 0.

Performance structure (TimelineSim ~44.1 us/core; cost model: matmul time =
out free-size x cycles/row; fp32r 1 cyc/row at N>=256; bf16 1 cyc/row any N):
  - conv1: 3-fold tap-stacked input (shifts 0/5/10, K=108) -> 5 matmuls/chunk.
  - conv2: chunk 0 unpacked (15 K=72 matmuls, gated only on pool1); chunks 1/2
    K-packed to 9 matmuls via p1s rows 72:128 (= p1 rows 0:56 shifted 8, one
    DMA) + st2 (16ch x taps 8-14 c-major stack). The packing DMAs can't land
    before ~13us (pool1 + DMA gen/sem latency), hence the hybrid.
  - conv3: 3D strided rhs [[105,3],[1,92]] trims phase padding (N=276/chunk);
    stacked-tail matmuls deferred to each chunk's end so the per-trio
    stka/stkb stack DMAs (gated on pool2 b-parts) never stall the PE.
  - S-stage: all-bf16 tiny matmuls (p3 and G in bf16) accumulate the final
    [4,27] per-phase partials in PSUM; one DVE reduce -> DMA out. The last
    tile's S-matmuls are split: the g0/g1-phase part runs before the final
    conv3 chunk's elu/pool chain, leaving only the 9 g2-phase columns on the
    tail critical path.
  - pstate discipline: matmul cost locks at SEQ-decode; a drained PE pipeline
    resets the clock ramp (next ~30 matmuls priced at 0.65/1.2 GHz). Filler
    groups sized to cover every dependency gap; 4 tiny x-gated shield matmuls
    occupy the wait-queue so conv1 prices at full clock.
  - DMA: HWDGE desc-gen is a shared serialized ~630ns/DMA device and the
    transfer pipe is a single 360GB/s FIFO, so inputs are packed into few
    tensors, w3 is split (taps 0-7 early / rest behind the stack pieces on
    the gpsimd/SWDGE queue), and x rides 3 parallel queues (SP/Act/Pool).

Sharding: data-parallel over batch; core i handles batch element i. No
collectives; host scatters x and gathers the 8 [4]-vectors.
"""
import numpy as np
import ml_dtypes

import concourse.bass as bass
import concourse.bacc as bacc
import concourse.mybir as mybir
import concourse.tile as tile
from concourse.bass_utils import run_bass_kernel_spmd

F32 = mybir.dt.float32
F32R = mybir.dt.float32r
BF16 = mybir.dt.bfloat16
AFT = mybir.ActivationFunctionType

EPS = 1e-5
B, C_IN, T, CROP, N_CROPS = 8, 36, 1001, 500, 501
N_CORES = 8

W1 = T - 14                       # 987 conv1 out cols
K1 = [(W1 - r) // 3 for r in range(3)]         # [329, 328, 328]
P1_PITCH = 329
OFF1 = [0, 329, 658]

W2 = [K1[r] - 14 for r in range(3)]            # [315, 314, 314]
C2_CHUNKS = [(0, 316), (329, 314), (658, 314)]  # phase-aligned, even N
K2 = [(W2[i2 // 3] - (i2 % 3)) // 3 for i2 in range(9)]
P2_PITCH = 105
P2_W = 9 * P2_PITCH               # 945
C2_W = 976

W3 = [K2[i2] - 14 for i2 in range(9)]
PH3 = 92                          # conv3 stored cols per i2-phase (incl 1 pad)
C3_CHUNK = 3 * PH3                # 276 cols per g-chunk matmul
C3_W = 3 * C3_CHUNK               # 828
K3 = [(W3[i3 // 3] - (i3 % 3)) // 3 for i3 in range(27)]
P3_PITCH = 30
P3_W = 27 * P3_PITCH              # 810
STACK_W = 932                     # conv3 stacked-rhs width


def _m_of_i3(i3):
    r1, r2, r3 = (i3 // 9), (i3 // 3) % 3, i3 % 3
    return 9 * r3 + 3 * r2 + r1


# class A: qmax=18 (19 crops); class B: qmax=17; i3=9 (m=1) excludes q=0
B_SET = sorted(i3 for i3 in range(27) if _m_of_i3(i3) >= 16)
assert B_SET == [2, 5, 8, 11, 14, 16, 17, 20, 23, 25, 26]
for i3 in range(27):
    q_need = (N_CROPS - _m_of_i3(i3)) // 27
    assert q_need + 11 <= K3[i3]


def _fv(tile_ap, rows, col0, dims):
    """Free-strided view: partition range + explicit [step,count] free dims."""
    base = tile_ap[rows[0]:rows[1], col0:col0 + 1]
    return bass.AP(base.tensor, base.offset, [list(base.ap[0])] + [list(d) for d in dims])


def build(mm_dtype=F32R, fillers=(6, 12, 0, 6)):
    nc = bacc.Bacc(None, target_bir_lowering=False, debug=False)

    # packed inputs: few DMAs (HWDGE desc-gen is a serialized ~630ns/DMA device)
    d_x = nc.dram_tensor("xb", [C_IN, T], mm_dtype, kind="ExternalInput")
    d_w1b = nc.dram_tensor("w1bt", [108, 5 * 72 + 1], mm_dtype, kind="ExternalInput")
    d_w2 = nc.dram_tensor("w2t", [128, 9 * 144], mm_dtype, kind="ExternalInput")
    d_w2tl = nc.dram_tensor("w2tlt", [72, 7 * 144], mm_dtype, kind="ExternalInput")
    d_w3p1 = nc.dram_tensor("w3p1t", [128, 8 * 288], mm_dtype, kind="ExternalInput")
    d_w3p2 = nc.dram_tensor("w3p2t", [128, 9 * 288], mm_dtype, kind="ExternalInput")
    d_spack = nc.dram_tensor("spt", [128, 5], F32, kind="ExternalInput")
    d_gpack = nc.dram_tensor("gpt", [128, 480], BF16, kind="ExternalInput")
    d_out = nc.dram_tensor("outd", [4, 1], F32, kind="ExternalOutput")

    def mm(out, lhsT, rhs, start, stop):
        nc.tensor.matmul(out, lhsT, rhs, start=start, stop=stop)

    with tile.TileContext(nc) as tc:
        with (
            tc.tile_pool(name="const", bufs=1) as cpool,
            tc.tile_pool(name="acts", bufs=1) as apool,
            tc.tile_pool(name="scratch", bufs=6) as spool,
            tc.tile_pool(name="psum", bufs=6, space="PSUM") as ppool,
            tc.tile_pool(name="psum_s", bufs=1, space="PSUM") as pspool,
        ):
            # ---- PE warm-up: ramp the clock while input DMAs run ----
            wt = cpool.tile([128, 256], mm_dtype, tag="wt")
            nc.vector.memset(wt[:].bitcast(F32), 0.0)
            wps = ppool.tile([128, 256], F32, tag="ps")
            for i in range(fillers[0]):
                nc.tensor.matmul(wps[:, 0:256], wt[:, 0:128], wt[:, 0:256],
                                 start=(i == 0), stop=(i == fillers[0] - 1))

            def filler(n_mm):
                if n_mm <= 0:
                    return
                fps = ppool.tile([128, 256], F32, tag="ps")
                for i in range(n_mm):
                    nc.tensor.matmul(fps[:, 0:256], wt[:, 0:128], wt[:, 0:256],
                                     start=(i == 0), stop=(i == n_mm - 1))

            # ---- input x, 3-fold tap stack: rows 36:72 = x+5, 72:108 = x+10
            # pad memsets are disjoint from the DMA targets so nothing blocks
            xs = cpool.tile([108, 1008], mm_dtype, tag="xs")
            nc.vector.memset(xs[0:108, 991:1008].bitcast(F32), 0.0)
            # x pieces go out on three different DMA queues in parallel
            nc.sync.dma_start(xs[0:36, 0:T], d_x[:])
            nc.scalar.dma_start(xs[36:72, 0:T - 5], d_x[:, 5:T])
            nc.gpsimd.dma_start(xs[72:108, 0:T - 10], d_x[:, 10:T])

            w1b = cpool.tile([108, 5 * 72 + 1], mm_dtype, tag="w1b")
            nc.sync.dma_start(w1b[:], d_w1b[:])
            w1s = w1b
            b1s = w1b[0:108, 360:361].bitcast(F32)
            spk = cpool.tile([128, 5], F32, tag="spk")
            nc.sync.dma_start(spk[:], d_spack[:])
            w2s = cpool.tile([128, 9 * 144], mm_dtype, tag="w2s")
            nc.sync.dma_start(w2s[:], d_w2[:])
            w2tl = cpool.tile([72, 7 * 144], mm_dtype, tag="w2tl")
            nc.sync.dma_start(w2tl[:], d_w2tl[:])
            b2a, b2b = spk[:, 0:1], spk[:, 1:2]
            b3a, b3b, b3c = spk[:, 2:3], spk[:, 3:4], spk[:, 4:5]
            w3p1 = cpool.tile([128, 8 * 288], mm_dtype, tag="w3p1")
            nc.sync.dma_start(w3p1[:], d_w3p1[:])
            w3p2 = cpool.tile([128, 9 * 288], mm_dtype, tag="w3p2")
            w3sa = w3p2[:, 7 * 288:8 * 288]
            w3sb = w3p2[0:112, 8 * 288:9 * 288]

            def w3blk(k, m0, ml):
                if k < 8:
                    return w3p1[:, k * 288 + m0:k * 288 + m0 + ml]
                return w3p2[:, (k - 8) * 288 + m0:(k - 8) * 288 + m0 + ml]

            gpk = cpool.tile([128, 480], BF16, tag="gpk")
            gaa, gab, gac = gpk[:, 0:116], gpk[:, 116:232], gpk[0:32, 232:348]
            gna, gnb, gnc = gpk[:, 348:392], gpk[:, 392:436], gpk[0:32, 436:480]

            full1 = apool.tile([72, 992], F32, tag="full1")
            p1s = apool.tile([128, 988], mm_dtype, tag="p1s")
            p1 = p1s[0:72, :]
            st2 = apool.tile([112, 988], mm_dtype, tag="st2")
            c2a = apool.tile([128, C2_W], F32, tag="c2a")
            c2b = apool.tile([16, C2_W], F32, tag="c2b")
            p2a = apool.tile([128, 948], mm_dtype, tag="p2a")
            p2b = apool.tile([16, 948], mm_dtype, tag="p2b")
            stka = apool.tile([128, STACK_W], mm_dtype, tag="stka")
            stkb = apool.tile([112, STACK_W], mm_dtype, tag="stkb")
            c3a = apool.tile([128, C3_W], F32, tag="c3a")
            c3b = apool.tile([128, C3_W], F32, tag="c3b")
            c3c = apool.tile([32, C3_W], F32, tag="c3c")
            p3a = apool.tile([128, P3_W], BF16, tag="p3a")
            p3b = apool.tile([128, P3_W], BF16, tag="p3b")
            p3c = apool.tile([32, P3_W], BF16, tag="p3c")
            red = apool.tile([4, 1], F32, tag="red")

            ps_s = pspool.tile([4, 27], F32, tag="pss")

            def elu1(ps_ap, rows, dst, dcol0, L, bias):
                """dst[:, dcol0:dcol0+L] = elu(ps + bias) + 1."""
                et = spool.tile([128, 512], F32, tag="et")
                d = dst[0:rows, dcol0:dcol0 + L]
                nc.scalar.activation(d, ps_ap, AFT.Relu, bias=bias[0:rows, 0:1])
                nc.scalar.activation(et[0:rows, 0:L], ps_ap, AFT.Exp,
                                     bias=bias[0:rows, 0:1])
                nc.vector.scalar_tensor_tensor(
                    d, et[0:rows, 0:L], 1.0, d,
                    op0=mybir.AluOpType.min, op1=mybir.AluOpType.add)

            nc.vector.memset(full1[:, 988:992], 0.0)
            nc.vector.memset(p1s[0:72, 987:988].bitcast(F32), 0.0)
            nc.vector.memset(p1s[0:72, 657:658].bitcast(F32), 0.0)

            # ---- wait-queue shield: 4 tiny x-gated matmuls absorb the early
            # (pre-ramp) cost lock-in so the real conv1 matmuls price warm.
            shps = ppool.tile([1, 2], F32, tag="ps")
            for i in range(4):
                nc.tensor.matmul(shps[:], xs[0:108, 0:1], xs[0:108, 0:2],
                                 start=(i == 0), stop=(i == 3))

            # ================= stage 1: conv1 [36 -> 72], 3-fold taps =========
            def conv1_chunk(n0, nl):
                ps = ppool.tile([72, 494], F32, tag="ps")
                for t in range(5):
                    mm(ps[:, 0:nl], w1s[:, t * 72:(t + 1) * 72],
                       xs[:, n0 + t:n0 + t + nl],
                       start=(t == 0), stop=(t == 4))
                elu1(ps[:, 0:nl], 72, full1, n0, nl, b1s)

            def pool1(r, k0, kn):
                a0 = _fv(full1[:], (0, 72), r + 0 + 3 * k0, [[3, kn]])
                a1 = _fv(full1[:], (0, 72), r + 1 + 3 * k0, [[3, kn]])
                a2 = _fv(full1[:], (0, 72), r + 2 + 3 * k0, [[3, kn]])
                o = p1[:, OFF1[r] + k0:OFF1[r] + k0 + kn]
                nc.vector.tensor_add(o, a0, a1)
                nc.vector.tensor_add(o, o, a2)

            def conv2_stack(a):
                # rows 72:128 of p1s = p1 rows 0:56 shifted by 8 (taps t+8);
                # st2 rows c*7+(j-8) = p1 row 56+c shifted by j (taps 8..14)
                c0, w = ((0, 316), (329, 314), (658, 314))[a]
                base = p1s[:]
                pitch = base.ap[0][0]
                nc.sync.dma_start(p1s[72:128, c0:c0 + w + 7],
                                  bass.AP(base.tensor, base.offset + 8 + c0,
                                          [[pitch, 56], [1, w + 7]]))
                nc.gpsimd.dma_start(st2[0:112, c0:c0 + w],
                                    bass.AP(base.tensor,
                                            base.offset + 56 * pitch + 8 + c0,
                                            [[pitch, 16], [1, 7], [1, w]]))

            def conv2_chunk(n0, nl, packed):
                if not isinstance(packed, tuple):
                    packed = (packed, packed)
                for (m0, ml, dst, bias), pk in zip(
                        ((0, 128, c2a, b2a), (128, 16, c2b, b2b)), packed):
                    ps = ppool.tile([128, 316], F32, tag="ps")
                    if pk:
                        for t in range(8):
                            mm(ps[0:ml, 0:nl], w2s[:, t * 144 + m0:t * 144 + m0 + ml],
                               p1s[:, n0 + t:n0 + t + nl], start=(t == 0), stop=False)
                        mm(ps[0:ml, 0:nl], w2s[0:112, 8 * 144 + m0:8 * 144 + m0 + ml],
                           st2[:, n0:n0 + nl], start=False, stop=True)
                    else:
                        for t in range(8):
                            mm(ps[0:ml, 0:nl], w2s[0:72, t * 144 + m0:t * 144 + m0 + ml],
                               p1s[0:72, n0 + t:n0 + t + nl], start=(t == 0), stop=False)
                        for t in range(7):
                            mm(ps[0:ml, 0:nl], w2tl[:, t * 144 + m0:t * 144 + m0 + ml],
                               p1s[0:72, n0 + 8 + t:n0 + 8 + t + nl],
                               start=False, stop=(t == 6))
                    elu1(ps[0:ml, 0:nl], ml, dst, n0, nl, bias)

            def pool2(r1):
                for (src, dst, rows) in ((c2a, p2a, 128), (c2b, p2b, 16)):
                    a0 = _fv(src[:], (0, rows), OFF1[r1] + 0, [[1, 3], [3, P2_PITCH]])
                    a1 = _fv(src[:], (0, rows), OFF1[r1] + 1, [[1, 3], [3, P2_PITCH]])
                    a2 = _fv(src[:], (0, rows), OFF1[r1] + 2, [[1, 3], [3, P2_PITCH]])
                    o = _fv(dst[:], (0, rows), 315 * r1, [[P2_PITCH, 3], [1, P2_PITCH]])
                    nc.vector.tensor_add(o, a0, a1)
                    nc.vector.tensor_add(o, o, a2)

            def stack_dma(c0, cl, b_cl):
                # stack rows r = c*8 + j (stka, taps j=0..7) / c*7 + j-8 (stkb):
                # one DMA per tile+piece; source AP leads with the partition dim.
                # stkb's first piece stops at 610 so it only needs pool2(0,1);
                # cols 610:617 are memset (they only feed the pad output col).
                pb = p2b[:]
                src_a = bass.AP(pb.tensor, pb.offset + c0, [[948, 16], [1, 8], [1, cl]])
                nc.sync.dma_start(stka[0:128, c0:c0 + cl], src_a)
                src_b = bass.AP(pb.tensor, pb.offset + c0 + 8, [[948, 16], [1, 7], [1, b_cl]])
                nc.sync.dma_start(stkb[0:112, c0:c0 + b_cl], src_b)

            C3M = ((0, 128, c3a, b3a), (128, 128, c3b, b3b), (256, 32, c3c, b3c))
            rhs3 = lambda base, off: bass.AP(
                base.tensor, base.offset + off,
                [list(base.ap[0]), [P2_PITCH, 3], [1, PH3]])

            def conv3_mains(g, mi, ps):
                (m0, ml, dst, bias) = C3M[mi]
                for k in range(15):
                    mm(ps[0:ml, 0:C3_CHUNK], w3blk(k, m0, ml),
                       rhs3(p2a[:], 315 * g + k), start=(k == 0), stop=False)

            def conv3_stks(g, mi, ps):
                (m0, ml, dst, bias) = C3M[mi]
                mm(ps[0:ml, 0:C3_CHUNK], w3sa[:, m0:m0 + ml],
                   rhs3(stka[:], 315 * g), start=False, stop=False)
                mm(ps[0:ml, 0:C3_CHUNK], w3sb[:, m0:m0 + ml],
                   rhs3(stkb[:], 315 * g), start=False, stop=True)

            def conv3_elu(g, mi, ps):
                (m0, ml, dst, bias) = C3M[mi]
                elu1(ps[0:ml, 0:C3_CHUNK], ml, dst, C3_CHUNK * g, C3_CHUNK, bias)

            def conv3_mtile(g, mi):
                ps = ppool.tile([128, C3_CHUNK], F32, tag="ps")
                conv3_mains(g, mi, ps)
                conv3_stks(g, mi, ps)
                conv3_elu(g, mi, ps)

            def pool3(g, ti, r0=0, rn=3):
                (src, dst, rows) = ((c3a, p3a, 128), (c3b, p3b, 128), (c3c, p3c, 32))[ti]
                gi = [[PH3, rn], [1, 3], [3, P3_PITCH]]
                go = [[3 * P3_PITCH, rn], [P3_PITCH, 3], [1, P3_PITCH]]
                base = C3_CHUNK * g + PH3 * r0
                a0 = _fv(src[:], (0, rows), base + 0, gi)
                a1 = _fv(src[:], (0, rows), base + 1, gi)
                a2 = _fv(src[:], (0, rows), base + 2, gi)
                o = _fv(dst[:], (0, rows), 270 * g + 90 * r0, go)
                nc.vector.tensor_add(o, a0, a1)
                nc.vector.tensor_add(o, o, a2)

            def s_mms(ti, part="all"):
                """Accumulate final [4,27] per-phase partials from p3 tile ti.
                part="early": only i3 0..17 (ready after pool3 g<=1);
                part="tail": only i3 18..26 (needs pool3(2, ti))."""
                (p3t, ga, gn, rows) = ((p3a, gaa, gna, 128), (p3b, gab, gnb, 128),
                                       (p3c, gac, gnc, 32))[ti]
                base = p3t[0:rows, :]
                rhs = lambda off, dims: bass.AP(
                    base.tensor, base.offset + off,
                    [list(base.ap[0])] + [list(d) for d in dims])
                early = part in ("all", "early")
                tail = part in ("all", "tail")
                # main: G_A, u = 0..28
                for u in range(29):
                    if part == "all":
                        mm(ps_s[:, 0:27], ga[0:rows, 4 * u:4 * u + 4],
                           rhs(u, [[30, 27]]), start=(ti == 0 and u == 0),
                           stop=False)
                    elif part == "early":
                        mm(ps_s[:, 0:18], ga[0:rows, 4 * u:4 * u + 4],
                           rhs(u, [[30, 18]]), start=False, stop=False)
                    else:
                        mm(ps_s[:, 18:27], ga[0:rows, 4 * u:4 * u + 4],
                           rhs(540 + u, [[30, 9]]), start=False, stop=False)
                # class-B correction: subtract wfc3[u-18] on B phases
                for u in range(18, 29):
                    j4 = 4 * (u - 18)
                    if early:
                        n1 = 9 if part == "all" else 6
                        mm(_fv(ps_s[:], (0, 4), 2, [[3, n1]]),
                           gn[0:rows, j4:j4 + 4], rhs(60 + u, [[90, n1]]),
                           start=False, stop=False)
                        mm(ps_s[:, 16:17], gn[0:rows, j4:j4 + 4],
                           p3t[0:rows, 480 + u:481 + u], start=False, stop=False)
                    if part == "early":
                        pass
                    if part == "tail":
                        mm(_fv(ps_s[:], (0, 4), 20, [[3, 3]]),
                           gn[0:rows, j4:j4 + 4], rhs(600 + u, [[90, 3]]),
                           start=False, stop=False)
                    if tail:
                        mm(ps_s[:, 25:26], gn[0:rows, j4:j4 + 4],
                           p3t[0:rows, 750 + u:751 + u], start=False,
                           stop=(part == "tail" and u == 28))
                # m=1 (i3=9) correction: subtract the q=0 crop
                if early:
                    for j in range(11):
                        mm(ps_s[:, 9:10], gn[0:rows, 4 * j:4 * j + 4],
                           p3t[0:rows, 270 + j:271 + j],
                           start=False, stop=(part == "all" and ti == 2 and j == 10))

            # ======================= schedule =======================
            conv1_chunk(0, 494)
            conv1_chunk(494, 494)
            pool1(0, 0, K1[0])
            pool1(1, 0, K1[1])
            pool1(2, 0, K1[2])
            conv2_stack(1)
            conv2_stack(2)
            nc.gpsimd.dma_start(w3p2[:], d_w3p2[:])
            nc.sync.dma_start(gpk[:], d_gpack[:])
            filler(fillers[1])
            conv2_chunk(*C2_CHUNKS[0], packed=False)
            conv2_chunk(*C2_CHUNKS[1], packed=True)
            conv2_chunk(*C2_CHUNKS[2], packed=True)
            for t_ in (c2a, c2b):
                nc.vector.memset(t_[:, 316:329], 0.0)
                nc.vector.memset(t_[:, 643:658], 0.0)
                nc.vector.memset(t_[:, 972:C2_W], 0.0)
            pool2(0)
            pool2(1)
            pool2(2)
            nc.vector.memset(p2a[:, P2_W:948].bitcast(F32), 0.0)
            nc.vector.memset(p2b[:, P2_W:948].bitcast(F32), 0.0)
            filler(fillers[2])
            nc.vector.memset(stkb[:, 301:302].bitcast(F32), 0.0)
            nc.vector.memset(stkb[:, 610:617].bitcast(F32), 0.0)
            stack_dma(0, 302, 301)
            stack_dma(315, 302, 295)
            stack_dma(617, STACK_W - 617, STACK_W - 617)
            for g in (0, 1):
                pss = []
                for mi in range(3):
                    psg = ppool.tile([128, C3_CHUNK], F32, tag="ps", name=f"psg{g}{mi}")
                    pss.append(psg)
                for mi in range(3):
                    conv3_mains(g, mi, pss[mi])
                for mi in range(3):
                    conv3_stks(g, mi, pss[mi])
                for mi in range(3):
                    conv3_elu(g, mi, pss[mi])
                    pool3(g, mi)
            # g=2: stagger S-matmul emission one m-tile behind so the PE never
            # waits on the elu->pool3 chain of the tile it just produced.
            conv3_mtile(2, 0)
            pool3(2, 0)
            conv3_mtile(2, 1)
            pool3(2, 1)
            s_mms(0)
            conv3_mtile(2, 2)
            pool3(2, 2)
            s_mms(1)
            s_mms(2, "early")
            filler(fillers[3])
            s_mms(2, "tail")
            nc.vector.reduce_sum(red[:], ps_s[:, 0:27], axis=mybir.AxisListType.X)
            nc.sync.dma_start(d_out[:], red[:])

    nc.compile()
    return nc


# ----------------------- host side -----------------------

def _fold_bn(w, b, g, be, m, v):
    s = g.astype(np.float64) / np.sqrt(v.astype(np.float64) + EPS)
    return w.astype(np.float64) * s[:, None, None], \
        (b.astype(np.float64) - m.astype(np.float64)) * s + be.astype(np.float64)


def prep_inputs(inputs):
    w1, b1 = _fold_bn(inputs['w1'][:, :, 0, :], inputs['b1'], inputs['g1'],
                      inputs['be1'], inputs['m1'], inputs['v1'])
    w2, b2 = _fold_bn(inputs['w2'][:, :, 0, :], inputs['b2'], inputs['g2'],
                      inputs['be2'], inputs['m2'], inputs['v2'])
    w3, b3 = _fold_bn(inputs['w3'][:, :, 0, :], inputs['b3'], inputs['g3'],
                      inputs['be3'], inputs['m3'], inputs['v3'])
    wfc = inputs['wfc'].astype(np.float64)
    bfc = inputs['bfc'].astype(np.float64)

    w2s = w2 / 3.0
    b2s = b2 - w2.sum((1, 2))
    w3f = w3 / 3.0
    b3s = b3 - w3.sum((1, 2))
    wfc3 = wfc.reshape(4, 288, 11) / 3.0
    Ko = bfc - wfc.reshape(4, 288, 11).sum((1, 2))

    f32 = lambda a: np.ascontiguousarray(a, np.float32)
    bf16 = lambda a: np.ascontiguousarray(a.astype(ml_dtypes.bfloat16))

    # conv1 3-fold weights: block t rows 0:36/36:72/72:108 = taps t/t+5/t+10
    w1p = np.zeros((108, 5 * 72), np.float64)
    for t in range(5):
        for di, d in enumerate((0, 5, 10)):
            w1p[36 * di:36 * di + 36, t * 72:(t + 1) * 72] = w1[:, :, t + d].T

    # conv3 stacked leftover weights, c-major rows: w3sa[c*8+j] = w3f[:,128+c,j]
    w3sa = np.zeros((128, 288), np.float64)
    w3sb = np.zeros((112, 288), np.float64)
    for c in range(16):
        for j in range(8):
            w3sa[c * 8 + j, :] = w3f[:, 128 + c, j]
        for j in range(8, 15):
            w3sb[c * 7 + (j - 8), :] = w3f[:, 128 + c, j]

    # S-stage folded FC weights: G_A[o,c,u] = sum_j wfc3[o,c,j] (q-window 0..18)
    GA = np.zeros((4, 288, 29), np.float64)
    for u in range(29):
        j0, j1 = max(0, u - 18), min(10, u)
        GA[:, :, u] = wfc3[:, :, j0:j1 + 1].sum(-1) / N_CROPS
    GN = -wfc3 / N_CROPS        # [4, 288, 11] correction weights

    def pack_lhsT(W, c0, cl):   # [4, 288, U] -> [cl, U*4]
        return W[:, c0:c0 + cl, :].transpose(1, 2, 0).reshape(cl, -1)

    # w1 + b1 ride in one tensor (b1 in the last column, f32 bits)
    w1b = np.zeros((108, 361), np.float32)
    w1b[:, 0:360] = w1p.astype(np.float32)
    w1b[0:72, 360] = b1.astype(np.float32)

    # conv2 packed weights: 8 window k-tiles (72ch tap t + 56ch tap t+8) + st2
    w2pk = np.zeros((128, 9 * 144), np.float64)
    for t in range(8):
        w2pk[0:72, t * 144:(t + 1) * 144] = w2s[:, :, t].T
        if t < 7:
            w2pk[72:128, t * 144:(t + 1) * 144] = w2s[:, 0:56, t + 8].T
    for c in range(16):
        for jj in range(7):
            w2pk[c * 7 + jj, 8 * 144:9 * 144] = w2s[:, 56 + c, 8 + jj]
    w2tl = np.zeros((72, 7 * 144), np.float64)
    for t in range(7):
        w2tl[:, t * 144:(t + 1) * 144] = w2s[:, :, t + 8].T

    # conv3 weights split into two DMAs: taps 0-7 early, 8-14 + stacks late
    w3main = w3f[:, 0:128, :].transpose(1, 2, 0).reshape(128, 15 * 288)
    w3pk1 = np.ascontiguousarray(w3main[:, 0:8 * 288])
    w3pk2 = np.zeros((128, 9 * 288), np.float64)
    w3pk2[:, 0:7 * 288] = w3main[:, 8 * 288:15 * 288]
    w3pk2[:, 7 * 288:8 * 288] = w3sa
    w3pk2[0:112, 8 * 288:9 * 288] = w3sb

    # per-partition bias columns: b2a, b2b, b3a, b3b, b3c
    spack = np.zeros((128, 5), np.float64)
    spack[0:128, 0] = b2s[0:128]
    spack[0:16, 1] = b2s[128:144]
    spack[0:128, 2] = b3s[0:128]
    spack[0:128, 3] = b3s[128:256]
    spack[0:32, 4] = b3s[256:288]

    gpack = np.zeros((128, 480), np.float64)
    gpack[:, 0:116] = pack_lhsT(GA, 0, 128)
    gpack[:, 116:232] = pack_lhsT(GA, 128, 128)
    gpack[0:32, 232:348] = pack_lhsT(GA, 256, 32)
    gpack[:, 348:392] = pack_lhsT(GN, 0, 128)
    gpack[:, 392:436] = pack_lhsT(GN, 128, 128)
    gpack[0:32, 436:480] = pack_lhsT(GN, 256, 32)

    common = {
        "w1bt": w1b,
        "w2t": f32(w2pk),
        "w2tlt": f32(w2tl),
        "w3p1t": f32(w3pk1),
        "w3p2t": f32(w3pk2),
        "spt": f32(spack),
        "gpt": bf16(gpack),
    }
    x = np.asarray(inputs['x'], np.float32)
    in_maps = []
    for c in range(N_CORES):
        m = dict(common)
        m["xb"] = np.ascontiguousarray(x[c, :, 0, :])
        in_maps.append(m)
    return in_maps, f32(Ko)


_NC_CACHE = {}


def run(inputs, mm_dtype=F32R, **kw):
    key = str(mm_dtype)
    if key not in _NC_CACHE:
        _NC_CACHE[key] = build(mm_dtype)
    nc = _NC_CACHE[key]
    in_maps, Ko = prep_inputs(inputs)
    res = run_bass_kernel_spmd(nc, in_maps, core_ids=list(range(N_CORES)), **kw)
    out = np.stack([r["outd"].reshape(4) for r in res.results]) + Ko[None, :]
    return out.astype(np.float32), res


def kernel(**inputs):
    out, _ = run(inputs)
    return out
